# revision 37
# baseline (speedup 1.0000x reference)
"""Trainium2 Bass kernel for DeformationTrackerBiFlowModel — G=7 single-matmul.

Reference math (per batch element b, per step t):
    x_t   = [prev_out (2), fin_t (3)]            (5,)
    h_t   = tanh(x_t @ W_rnn + b_rnn)            (12,)   (U_rnn is inert)
    out_t = [cp0 (2), h_t (12)] @ W_out + b_out  (2,)
    prev_out_{t+1} = out_t;  prev_out_0 = cp0

One matmul + one tanh + one psum->sbuf copy per step per chain. With G=7
trajectories packed block-diagonally, the contraction stacks h (12G=84 rows)
and the input block (fin 3G=21 + ones 1 + cp0 2G=14 rows): K=120 <= 128.
The output packs pre (84) | pad (12) | out (14): M=110 <= 128, with the out
region at psum base partition 96 (legal engine AP base). Per step:

    psum_t[0:84]   = h_{t-1} @ Wh + [fin_t, 1, cp0] @ (wF part)   -> tanh
    psum_t[96:110] = h_{t-1} @ Wo2 + cp0 @ Wo1 + b_out            == out_{t-1}

(cvec = cp0@Wo1 + b_out is folded into the weight tile's out columns, so the
DVE pass is a pure copy+bf16-cast into the 4-step staging tile.)  tanh
writes h_t directly into the next step's rhs block.  There is no step-0
weight variant: block 0's h rows are initialized to
h_init = (cp0 - cvec) @ pinv(Wo2), which makes the steady-state weights
exact at t=0 (h_init @ Wo2 + cvec == cp0 == prev_out_0).

Steady state is ACT-bound at ~1452 ns/round (3 chains x ~484 ns tanh slot:
391 cols @1.2GHz + ~160 ns psum-access/seq overhead).  Pairing chains into
2-bank-wide ACT APs was tried and LOSES: the fat ACT feeds two matmuls
whose serial PE slots re-enter the next fat ACT's wait (ACT+2MM+2sem
~1.6-2.1 us/round), and the Tile scheduler serializes same-tile PSUM
readers in program order, which puts the out-copy behind the last tanh.

Setup: per-chain block-0 rhs (xinit0, h_init rows included) + ws first on
separate queues, then xinitR split by first-use deadline (block 1, blocks
2-3, blocks 4-7).  fin groups of 4 steps stream in ~5 steps ahead (21
descriptors per DMA); out groups of 4 steps stream out (14 descriptors).
The final group is drained in halves on the two emptiest queues, and no
fin prefetch is issued past the last real step.

Batch 65536 over 8 cores; per core G*C*COLS = 7*3*391 = 8211 (8192 + pad 19).
"""

import os
from contextlib import ExitStack

import numpy as np

import concourse.mybir as mybir
import concourse.tile as tile
from concourse import bacc
from concourse.bass_utils import run_bass_kernel_spmd

B, T = 65536, 100
D_CP, D_FIN, HID = 2, 3, 12
NCORES = 8
BC = B // NCORES              # 8192 per core
G = 7                         # trajectories packed per matmul (block-diag)
C = 3                         # independent column chains
COLS = 391                    # batch columns per chain
BP = G * C * COLS             # 8211 padded batch per core
NH = HID * G                  # 84: h rows (rhs) / pre rows (psum)
NFIN = D_FIN * G              # 21 fin rows
NCONST = 1 + D_CP * G         # 15: ones + cp0 rows
KTOT = NH + NFIN + NCONST     # 120
MOUT = D_CP * G               # 14 out rows
MPAD = 96                     # out region starts at psum partition 96
MTOT = MPAD + MOUT            # 110
NFG = (T + 3) // 4 + 1        # 26 fin groups (steps 4g..4g+3, zero padded)

F32 = mybir.dt.float32

_MM_CHOICES = {"bf16": mybir.dt.bfloat16, "f32r": mybir.dt.float32r, "f32": F32}
MM_DTYPE = _MM_CHOICES[os.environ.get("DTB_MM", "bf16")]
MM_NP = mybir.dt.np(MM_DTYPE)

LAST_RESULTS = None  # test.py introspects profiling info from here


def build_program(t_steps=T, g=G, c=C, cols=COLS, mm_dtype=None):
    if mm_dtype is None:
        mm_dtype = MM_DTYPE
    XDT = mm_dtype
    nh, nfin, nconst = HID * g, D_FIN * g, 1 + D_CP * g
    ktot = nh + nfin + nconst
    mout = D_CP * g
    mpad, mtot = MPAD, MPAD + D_CP * g
    nfg = (t_steps + 3) // 4 + 1
    nog = t_steps // 4
    nc = bacc.Bacc(target_bir_lowering=False)

    fin = nc.dram_tensor("fin", [c, nfin, nfg, 4 * cols], XDT, kind="ExternalInput")
    # xinit0 = full rhs block 0 (h_init + fin g0s0 + ones + cp0); xinitR =
    # rows nh: for blocks 1-7.
    xinit0 = nc.dram_tensor("xinit0", [c, ktot, cols], XDT, kind="ExternalInput")
    xinitR = nc.dram_tensor(
        "xinitR", [c, nfin + nconst, 7 * cols], XDT, kind="ExternalInput"
    )
    w = nc.dram_tensor("w", [ktot, mtot], XDT, kind="ExternalInput")
    out = nc.dram_tensor("out", [nog, c, mout, 4 * cols], XDT, kind="ExternalOutput")

    tanh = mybir.ActivationFunctionType.Tanh

    with tile.TileContext(nc) as tc, ExitStack() as ctx:
        const = ctx.enter_context(tc.tile_pool(name="const", bufs=1))
        xpool = ctx.enter_context(tc.tile_pool(name="xpool", bufs=1))
        opool = ctx.enter_context(tc.tile_pool(name="opool", bufs=3))
        psum = ctx.enter_context(tc.tile_pool(name="psum", bufs=2, space="PSUM"))

        # Critical path to the first matmul: ws first on sync (the earliest-
        # starting queue) + per-chain full block 0, one DMA per queue
        # (h_init rows make the steady-state weights exact at t=0, so there
        # is no w0 variant).
        ws = const.tile([ktot, mtot], XDT, name="ws")
        qs = [nc.gpsimd, nc.scalar, nc.sync]
        nc.sync.dma_start(out=ws, in_=w[:, :])
        xtiles = []
        for ch in range(c):
            xt = xpool.tile([ktot, 8 * cols], XDT, tag=f"x{ch}", name=f"x_{ch}")
            qs[ch].dma_start(out=xt[:, 0:cols], in_=xinit0[ch][:, :])
            xtiles.append(xt)
        # xinitR split by first-use step: block 1 (t=1), blocks 2-3, blocks
        # 4-7 — each queue issues them in deadline order.
        for lo, hi in ((0, 1), (1, 3), (3, 7)):
            for ch in range(c):
                qs[ch].dma_start(
                    out=xtiles[ch][nh:, (lo + 1) * cols : (hi + 1) * cols],
                    in_=xinitR[ch][:, lo * cols : hi * cols],
                )

        # fin prefetch on sync (ch 0/1) + gpsimd (ch 2); out stores on
        # gpsimd (ch 0/1) + sync (ch 2) so neither queue owns both flows.
        # The final-group halves alternate between scalar (whose DMA ring is
        # nearly empty, so the end-of-program drain clears it quickly — the
        # ACT engine is done by then) and sync, so their issue costs overlap.
        finq = [nc.sync, nc.sync, nc.gpsimd]
        outq = [nc.gpsimd, nc.gpsimd, nc.sync]
        lastq = {(0, 1): nc.scalar, (0, 3): nc.sync, (1, 1): nc.sync,
                 (1, 3): nc.scalar, (2, 1): nc.scalar, (2, 3): nc.sync}

        ostages = [None] * c
        for t in range(t_steps + 1):
            for ch in range(c):
                xt = xtiles[ch]
                blk = t % 8
                p1 = psum.tile([mtot, cols], F32, tag=f"p{ch}", name=f"p_{ch}_{t}")
                nc.tensor.matmul(
                    p1, ws,
                    xt[:, blk * cols : (blk + 1) * cols], start=True, stop=True,
                )

                if t < t_steps:
                    # h_t goes straight into the next step's rhs block, as
                    # two half-width tanhs so the second one's PSUM-access
                    # latency can pipeline under the first one's columns.
                    nb = (t + 1) % 8
                    hf = cols // 2
                    nc.scalar.activation(
                        xt[0:nh, nb * cols : nb * cols + hf],
                        p1[0:nh, 0:hf], tanh,
                    )
                    nc.scalar.activation(
                        xt[0:nh, nb * cols + hf : (nb + 1) * cols],
                        p1[0:nh, hf:], tanh,
                    )
                if t > 0:
                    ob = (t - 1) % 4
                    if ob == 0:
                        ostages[ch] = opool.tile(
                            [mout, 4 * cols], XDT, tag=f"o{ch}", name=f"o_{ch}_{t}"
                        )
                    ost = ostages[ch]
                    if t == t_steps and ch < 2:
                        # Final round: tanh is done, so the idle ACT engine
                        # absorbs two of the three trailing out copies while
                        # the DVE takes the third.
                        nc.scalar.activation(
                            ost[:, ob * cols : (ob + 1) * cols],
                            p1[mpad:mtot, :],
                            mybir.ActivationFunctionType.Copy,
                        )
                    else:
                        nc.vector.tensor_copy(
                            out=ost[:, ob * cols : (ob + 1) * cols],
                            in_=p1[mpad:mtot, :],
                        )
                    final = t > t_steps - 4
                    if final and ob == 1:
                        # Drain the last group in halves so the tail DMA is
                        # small and mostly overlapped with the closing steps.
                        lastq[ch, 1].dma_start(
                            out=out[(t - 2) // 4, ch][:, : 2 * cols],
                            in_=ost[:, : 2 * cols],
                        )
                    elif ob == 3:
                        if final:
                            lastq[ch, 3].dma_start(
                                out=out[(t - 4) // 4, ch][:, 2 * cols :],
                                in_=ost[:, 2 * cols :],
                            )
                        else:
                            outq[ch].dma_start(out=out[(t - 4) // 4, ch], in_=ost)
                # Prefetch fin group g = steps 4g..4g+3, ~5 steps ahead.
                # Groups past the last real step are never prefetched: the
                # t=t_steps matmul only reuses stale fin rows whose products
                # land in discarded pre rows.
                if t % 4 == 3:
                    gg = (t + 5) // 4
                    if 4 * gg < t_steps:
                        b0 = (4 * gg) % 8
                        finq[ch].dma_start(
                            out=xt[nh : nh + nfin, b0 * cols : (b0 + 4) * cols],
                            in_=fin[ch, :, gg, :],
                        )
    if os.environ.get("DTB_SKIPLDW"):
        _skip_repeat_ldweights(nc)
    nc.compile()
    return nc


def _skip_repeat_ldweights(nc):
    """Delete LDWEIGHTS whose weights AP matches the previous one in the PE
    stream (the array still holds those weights).  Waits are consolidated
    onto matmuls/ldweights first so only wait-free repeats are dropped."""
    nc.move_matmul_waits_to_ldweights()
    nc.generate_event_semaphores()
    removed = 0
    for blk in nc.main_func.blocks:
        insts = blk.instructions
        prev_key = None
        idx = 0
        while idx < len(insts):
            inst = insts[idx]
            if isinstance(inst, mybir.InstLdweights):
                key = str(inst.ins[0])
                si = inst.sync_info
                nowait = si is None or (
                    len(si.on_wait) == 0 and len(si.on_update) == 0
                )
                if key == prev_key and nowait:
                    del insts[idx]
                    removed += 1
                    continue
                prev_key = key
            idx += 1
    print(f"skip_ldweights: removed {removed} weight loads")


def build_packed_weights(W_rnn, W_out, b_rnn, b_out, g=G):
    W_rnn = np.asarray(W_rnn, np.float32)
    W_out = np.asarray(W_out, np.float32)
    b_rnn = np.asarray(b_rnn, np.float32)
    b_out = np.asarray(b_out, np.float32)
    W1p, W1f = W_rnn[:D_CP], W_rnn[D_CP:]
    Wo1, Wo2 = W_out[:D_CP], W_out[D_CP:]
    nh, nfin = HID * g, D_FIN * g
    ktot = nh + nfin + 1 + D_CP * g
    mpad, mtot = MPAD, MPAD + D_CP * g
    ones_row = nh + nfin
    cp0_base = ones_row + 1

    w = np.zeros((ktot, mtot), np.float32)
    w0 = np.zeros((ktot, mtot), np.float32)
    E = Wo1 @ W1p                      # (2, 12) cp0 contribution to next pre
    r = b_rnn + b_out @ W1p            # (12,) ones-row weight (steady state)
    Wh = Wo2 @ W1p                     # (12, 12) h contribution to next pre
    for i in range(g):
        hsl = slice(HID * i, HID * (i + 1))
        osl = slice(mpad + D_CP * i, mpad + D_CP * (i + 1))
        w[hsl, hsl] = Wh
        w[hsl, osl] = Wo2
        w0[hsl, osl] = Wo2             # unused at t=0 (h rows are zero) but harmless
        fsl = slice(nh + D_FIN * i, nh + D_FIN * (i + 1))
        w[fsl, hsl] = W1f
        w0[fsl, hsl] = W1f
        w[ones_row, hsl] = r
        w0[ones_row, hsl] = b_rnn
        # cvec = cp0 @ Wo1 + b_out folded into the out columns so psum[96:110]
        # holds the final out and the DVE pass is a pure copy+cast.
        w[ones_row, osl] = b_out
        w0[ones_row, osl] = b_out
        csl = slice(cp0_base + D_CP * i, cp0_base + D_CP * (i + 1))
        w[csl, hsl] = E
        w0[csl, hsl] = W1p
        w[csl, osl] = Wo1
        w0[csl, osl] = Wo1
    return w, w0


def stage_inputs(cp0, fin, h_init, g=G, c=C, cols=COLS, t_steps=T):
    """Batch-major -> feature-major device layouts (b = ch*(g*cols)+gi*cols+j)."""
    bp = g * c * cols
    bc = cp0.shape[0]
    nfg = (t_steps + 3) // 4 + 1
    F = np.zeros((bp, 4 * nfg, D_FIN), np.float32)
    F[:bc, :t_steps] = fin
    cp0_p = np.zeros((bp, D_CP), np.float32)
    cp0_p[:bc] = cp0
    hi_p = np.zeros((bp, HID), np.float32)
    hi_p[:bc] = h_init
    # fin group g covers steps 4g..4g+3, contiguous per partition row.
    fin_d = np.ascontiguousarray(
        F.reshape(c, g, cols, nfg, 4, D_FIN).transpose(0, 1, 5, 3, 4, 2)
    ).reshape(c, D_FIN * g, nfg, 4 * cols)
    nh, nfin, nconst = HID * g, D_FIN * g, 1 + D_CP * g
    cp0_rows = cp0_p.reshape(c, g, cols, D_CP).transpose(0, 1, 3, 2).reshape(
        c, D_CP * g, cols
    )
    h_rows = hi_p.reshape(c, g, cols, HID).transpose(0, 1, 3, 2).reshape(
        c, nh, cols
    )
    x0 = np.ones((c, nh + nfin + nconst, cols), np.float32)
    x0[:, :nh] = h_rows
    x0[:, nh : nh + nfin] = fin_d[:, :, 0, 0:cols]
    x0[:, nh + nfin + 1 :] = cp0_rows
    xr = np.ones((c, nfin + nconst, 7 * cols), np.float32)
    xr[:, :nfin] = fin_d[:, :, 0:2, :].reshape(c, nfin, 8 * cols)[:, :, cols:]
    xr[:, nfin + 1 :] = np.tile(cp0_rows, (1, 1, 7))
    return fin_d, x0, xr


def unstage_output(out_d, bc, g=G, c=C, cols=COLS, t_steps=T):
    """out_d [T/4, c, 14, 4*cols] (step-within-group along the row) -> batch."""
    bp = g * c * cols
    nog = t_steps // 4
    o = out_d.reshape(nog, c, g, D_CP, 4, cols).transpose(1, 2, 5, 0, 4, 3)
    return np.ascontiguousarray(o).reshape(bp, t_steps, D_CP)[:bc]


# ---------------------------------------------------------------------------
# v2: C=4 chains, rotating ACT pairs (2-bank PSUM APs), single quad-bank DVE
# copy per step, one 8-bank PSUM mega tile, no w0 (h rows of block 0 start at
# h_init = (cp0 - cvec) @ pinv(Wo2) so the steady-state weights are exact at
# t=0 too).
#
# Bank map per round t: even t -> banks 0-3 hold chains (0,1,2,3); odd t ->
# banks 4-7 hold chains (0,2,1,3).  ACT pair X always reads banks
# (base,base+1), Y (base+2,base+3); each next-round ACT pair waits on two
# matmuls fed by two DIFFERENT prior ACTs, so the serial cycle is
# ACT+1*MM+2*sem instead of ACT+2*MM+2*sem.
# rhs block layout: even blocks [ch0|ch2|ch1|ch3], odd blocks [ch0|ch1|ch2|ch3]
# (block parity == step parity), which makes every ACT output a contiguous
# half-block.
# ---------------------------------------------------------------------------

C2 = 4
COLS2 = 293                   # 7*4*293 = 8204 >= 8192
W2 = C2 * COLS2               # 1172 block width
BP2 = G * C2 * COLS2
PERM_EVEN = [0, 2, 1, 3]      # chain occupying each slot, even blocks
PERM_ODD = [0, 1, 2, 3]


def build_program2(t_steps=T, mm_dtype=None):
    if mm_dtype is None:
        mm_dtype = MM_DTYPE
    XDT = mm_dtype
    g, cols = G, COLS2
    nh, nfin, nconst = HID * g, D_FIN * g, 1 + D_CP * g
    ktot = nh + nfin + nconst
    mout = D_CP * g
    mpad, mtot = MPAD, MPAD + mout
    nfg = (t_steps + 3) // 4 + 1
    nog = t_steps // 4
    nc = bacc.Bacc(target_bir_lowering=False)

    fin = nc.dram_tensor("fin", [nfin, nfg, 4 * W2], XDT, kind="ExternalInput")
    xinit0 = nc.dram_tensor("xinit0", [ktot, W2], XDT, kind="ExternalInput")
    xinitR = nc.dram_tensor(
        "xinitR", [nfin + nconst, 7 * W2], XDT, kind="ExternalInput"
    )
    w = nc.dram_tensor("w", [ktot, mtot], XDT, kind="ExternalInput")
    out = nc.dram_tensor("out", [nog, mout, 4 * W2], XDT, kind="ExternalOutput")

    tanh = mybir.ActivationFunctionType.Tanh

    with tile.TileContext(nc) as tc, ExitStack() as ctx:
        const = ctx.enter_context(tc.tile_pool(name="const", bufs=1))
        xpool = ctx.enter_context(tc.tile_pool(name="xpool", bufs=1))
        opool = ctx.enter_context(tc.tile_pool(name="opool", bufs=3))
        psum = ctx.enter_context(tc.tile_pool(name="psum", bufs=1, space="PSUM"))

        ws = const.tile([ktot, mtot], XDT, name="ws")
        xt = xpool.tile([ktot, 8 * W2], XDT, name="xt")
        # Separate 4-bank tiles for even/odd rounds keep the dependency
        # tracker's ranges from uniting across parities.
        pmA = psum.tile([mtot, 2048], F32, name="pmA")
        pmB = psum.tile([mtot, 2048], F32, name="pmB")

        # Setup: 4 DMAs. Critical: block 0 (sync) + weights (gpsimd);
        # blocks 1-3 next (needed at t=1), 4-7 last (scalar starts late
        # behind its ACT_TABLE_LOAD anyway).
        nc.sync.dma_start(out=xt[:, 0:W2], in_=xinit0[:, :])
        nc.gpsimd.dma_start(out=ws, in_=w[:, :])
        nc.gpsimd.dma_start(
            out=xt[nh:, W2 : 4 * W2], in_=xinitR[:, 0 : 3 * W2]
        )
        nc.scalar.dma_start(out=xt[nh:, 4 * W2 :], in_=xinitR[:, 3 * W2 :])

        pm3A = pmA[:, :].rearrange("p (b q) -> p b q", q=512)
        pm3B = pmB[:, :].rearrange("p (b q) -> p b q", q=512)

        ostage = None
        for t in range(t_steps + 1):
            even = t % 2 == 0
            pm = pmA if even else pmB
            pm3 = pm3A if even else pm3B
            bank_ch = PERM_ODD if even else PERM_EVEN   # bank slot -> chain
            pblk = PERM_EVEN if even else PERM_ODD      # rhs block slot -> chain
            rs = [pblk.index(bank_ch[s]) for s in range(4)]
            blk = t % 8
            nb = (t + 1) % 8

            def mm(s):
                nc.tensor.matmul(
                    pm[:, 512 * s : 512 * s + cols],
                    ws,
                    xt[:, blk * W2 + rs[s] * cols : blk * W2 + (rs[s] + 1) * cols],
                    start=True, stop=True,
                )

            def act(half):
                nc.scalar.activation(
                    xt[0:nh, nb * W2 + half * 2 * cols : nb * W2 + (half + 1) * 2 * cols]
                    .rearrange("p (b q) -> p b q", q=cols),
                    pm3[0:nh, 2 * half : 2 * half + 2, 0:cols],
                    tanh,
                )

            mm(0)
            mm(1)
            if t < t_steps:
                act(0)
            mm(2)
            mm(3)
            if t < t_steps:
                act(1)
            if t > 0:
                ob = (t - 1) % 4
                if ob == 0:
                    ostage = opool.tile([mout, 4 * W2], XDT, tag="o", name=f"o_{t}")
                nc.vector.tensor_copy(
                    out=ostage[:, ob * W2 : (ob + 1) * W2]
                    .rearrange("p (b q) -> p b q", q=cols),
                    in_=pm3[mpad:mtot, 0:4, 0:cols],
                )
                final = t > t_steps - 4
                if final and ob == 1:
                    nc.scalar.dma_start(
                        out=out[(t - 2) // 4][:, : 2 * W2],
                        in_=ostage[:, : 2 * W2],
                    )
                elif ob == 3:
                    if final:
                        nc.scalar.dma_start(
                            out=out[(t - 4) // 4][:, 2 * W2 :],
                            in_=ostage[:, 2 * W2 :],
                        )
                    else:
                        nc.gpsimd.dma_start(out=out[(t - 4) // 4], in_=ostage)
            if t % 4 == 3:
                gg = (t + 5) // 4
                if 4 * gg < t_steps:
                    b0 = (4 * gg) % 8
                    nc.sync.dma_start(
                        out=xt[nh : nh + nfin, b0 * W2 : (b0 + 4) * W2],
                        in_=fin[:, gg, :],
                    )
    if os.environ.get("DTB_SKIPLDW"):
        _skip_repeat_ldweights(nc)
    nc.compile()
    return nc


def stage_inputs2(cp0, fin, h_init, t_steps=T):
    """Per-core staging for v2 (chain-rotated block layouts)."""
    g, c, cols = G, C2, COLS2
    bp, bc = BP2, cp0.shape[0]
    nfg = (t_steps + 3) // 4 + 1
    F = np.zeros((bp, 4 * nfg, D_FIN), np.float32)
    F[:bc, :t_steps] = fin
    cp0_p = np.zeros((bp, D_CP), np.float32)
    cp0_p[:bc] = cp0
    hi_p = np.zeros((bp, HID), np.float32)
    hi_p[:bc] = h_init

    # feature-major per chain: [c, rows, ..., cols]
    ffeat = F.reshape(c, g, cols, nfg, 4, D_FIN).transpose(0, 1, 5, 3, 4, 2)
    ffeat = np.ascontiguousarray(ffeat).reshape(c, D_FIN * g, nfg, 4, cols)
    cfeat = cp0_p.reshape(c, g, cols, D_CP).transpose(0, 1, 3, 2).reshape(
        c, D_CP * g, cols
    )
    hfeat = hi_p.reshape(c, g, cols, HID).transpose(0, 1, 3, 2).reshape(
        c, HID * g, cols
    )

    nfin = D_FIN * g
    fin_d = np.zeros((nfin, nfg, 4, c * cols), np.float32)
    for s in range(4):
        perm = PERM_EVEN if s % 2 == 0 else PERM_ODD
        for slot in range(c):
            fin_d[:, :, s, slot * cols : (slot + 1) * cols] = ffeat[
                perm[slot], :, :, s, :
            ]
    cp_rows = np.zeros((2, D_CP * g, c * cols), np.float32)  # [parity]
    for par in range(2):
        perm = PERM_EVEN if par == 0 else PERM_ODD
        for slot in range(c):
            cp_rows[par, :, slot * cols : (slot + 1) * cols] = cfeat[perm[slot]]

    ktot = HID * g + nfin + 1 + D_CP * g
    x0 = np.ones((ktot, c * cols), np.float32)
    for slot in range(c):
        x0[: HID * g, slot * cols : (slot + 1) * cols] = hfeat[PERM_EVEN[slot]]
    x0[HID * g : HID * g + nfin] = fin_d[:, 0, 0, :]
    x0[HID * g + nfin + 1 :] = cp_rows[0]

    xr = np.ones((nfin + 1 + D_CP * g, 7, c * cols), np.float32)
    for b in range(1, 8):
        xr[:nfin, b - 1] = fin_d[:, b // 4, b % 4, :]
        xr[nfin + 1 :, b - 1] = cp_rows[b % 2]
    xr = xr.reshape(-1, 7 * c * cols)
    fin_d = fin_d.reshape(nfin, nfg, 4 * c * cols)
    return fin_d, x0, xr


def unstage_output2(out_d, bc, t_steps=T):
    """out_d [nog, 14, 4, c*cols] -> [bc, T, 2].  Sub-block ob holds step
    tau=4q+ob computed at round t=tau+1; chain order is identity for odd tau,
    [0,2,1,3] for even tau."""
    g, c, cols = G, C2, COLS2
    nog = t_steps // 4
    o = out_d.reshape(nog, D_CP * g, 4, c, cols)
    res = np.empty((c, g, cols, nog, 4, D_CP), np.float32)
    for ob in range(4):
        perm = PERM_EVEN if ob % 2 == 0 else PERM_ODD
        for slot in range(c):
            # o[q, 2*gi+d, ob, slot, j] -> chain perm[slot]
            res[perm[slot], :, :, :, ob, :] = o[:, :, ob, slot, :].reshape(
                nog, g, D_CP, cols
            ).transpose(1, 3, 0, 2)
    return res.reshape(BP2, t_steps, D_CP)[:bc]


def kernel(control_point_input, finger_input, W_rnn, U_rnn, b_rnn, W_out, b_out):
    global LAST_RESULTS
    cp = np.asarray(control_point_input, np.float32)
    fin = np.asarray(finger_input, np.float32)
    W_rnn = np.asarray(W_rnn, np.float32)
    b_rnn = np.asarray(b_rnn, np.float32)
    W_out = np.asarray(W_out, np.float32)
    b_out = np.asarray(b_out, np.float32)

    cp0 = cp[:, 0, :]
    w, _ = build_packed_weights(W_rnn, W_out, b_rnn, b_out)
    w = w.astype(MM_NP)
    # h_init makes the steady-state weights exact at t=0 (no w0 pass needed):
    # h_init @ Wo2 = cp0 - cvec reproduces out_{-1} = cp0 through the packed
    # recurrence.
    Wo2 = W_out[D_CP:]
    cvec = cp0 @ W_out[:D_CP] + b_out
    h_init = (cp0 - cvec) @ np.linalg.pinv(Wo2)

    trace = bool(os.environ.get("DTB_TRACE"))
    if os.environ.get("DTB_V2"):
        nc = build_program2()
        in_maps = []
        for m in range(NCORES):
            sl = slice(m * BC, (m + 1) * BC)
            fin_d, x0_d, xr_d = stage_inputs2(cp0[sl], fin[sl], h_init[sl])
            in_maps.append(
                {"fin": fin_d.astype(MM_NP, copy=False),
                 "xinit0": x0_d.astype(MM_NP, copy=False),
                 "xinitR": xr_d.astype(MM_NP, copy=False), "w": w}
            )
        res = run_bass_kernel_spmd(
            nc, in_maps, core_ids=list(range(NCORES)), trace=trace
        )
        LAST_RESULTS = res
        outs = [
            unstage_output2(np.asarray(res.results[m]["out"], np.float32), BC)
            for m in range(NCORES)
        ]
        return np.concatenate(outs, axis=0)

    nc = build_program()
    in_maps = []
    for m in range(NCORES):
        sl = slice(m * BC, (m + 1) * BC)
        fin_d, x0_d, xr_d = stage_inputs(cp0[sl], fin[sl], h_init[sl])
        in_maps.append(
            {"fin": fin_d.astype(MM_NP, copy=False),
             "xinit0": x0_d.astype(MM_NP, copy=False),
             "xinitR": xr_d.astype(MM_NP, copy=False), "w": w}
        )
    res = run_bass_kernel_spmd(
        nc, in_maps, core_ids=list(range(NCORES)), trace=trace
    )
    LAST_RESULTS = res
    outs = [
        unstage_output(np.asarray(res.results[m]["out"], np.float32), BC)
        for m in range(NCORES)
    ]
    return np.concatenate(outs, axis=0)



# revision 38
# speedup vs baseline: 1.2996x; 1.2996x over previous
"""Trainium2 Bass kernel for DeformationTrackerBiFlowModel — G=7 single-matmul.

Reference math (per batch element b, per step t):
    x_t   = [prev_out (2), fin_t (3)]            (5,)
    h_t   = tanh(x_t @ W_rnn + b_rnn)            (12,)   (U_rnn is inert)
    out_t = [cp0 (2), h_t (12)] @ W_out + b_out  (2,)
    prev_out_{t+1} = out_t;  prev_out_0 = cp0

One matmul + one tanh + one psum->sbuf copy per step per chain. With G=7
trajectories packed block-diagonally, the contraction stacks h (12G=84 rows)
and the input block (fin 3G=21 + ones 1 + cp0 2G=14 rows): K=120 <= 128.
The output packs pre (84) | pad (12) | out (14): M=110 <= 128, with the out
region at psum base partition 96 (legal engine AP base). Per step:

    psum_t[0:84]   = h_{t-1} @ Wh + [fin_t, 1, cp0] @ (wF part)   -> tanh
    psum_t[96:110] = h_{t-1} @ Wo2 + cp0 @ Wo1 + b_out            == out_{t-1}

(cvec = cp0@Wo1 + b_out is folded into the weight tile's out columns, so the
DVE pass is a pure copy+bf16-cast into the 4-step staging tile.)  tanh
writes h_t directly into the next step's rhs block.  There is no step-0
weight variant: block 0's h rows are initialized to
h_init = (cp0 - cvec) @ pinv(Wo2), which makes the steady-state weights
exact at t=0 (h_init @ Wo2 + cvec == cp0 == prev_out_0).

Steady state is ACT-bound at ~1452 ns/round (3 chains x ~484 ns tanh slot:
391 cols @1.2GHz + ~160 ns psum-access/seq overhead).  Pairing chains into
2-bank-wide ACT APs was tried and LOSES: the fat ACT feeds two matmuls
whose serial PE slots re-enter the next fat ACT's wait (ACT+2MM+2sem
~1.6-2.1 us/round), and the Tile scheduler serializes same-tile PSUM
readers in program order, which puts the out-copy behind the last tanh.

Setup: per-chain block-0 rhs (xinit0, h_init rows included) + ws first on
separate queues, then xinitR split by first-use deadline (block 1, blocks
2-3, blocks 4-7).  fin groups of 4 steps stream in ~5 steps ahead (21
descriptors per DMA); out groups of 4 steps stream out (14 descriptors).
The final group is drained in halves on the two emptiest queues, and no
fin prefetch is issued past the last real step.

Batch 65536 over 8 cores; per core G*C*COLS = 7*3*391 = 8211 (8192 + pad 19).
"""

import os
from contextlib import ExitStack

import numpy as np

import concourse.mybir as mybir
import concourse.tile as tile
from concourse import bacc
from concourse.bass_utils import run_bass_kernel_spmd

B, T = 65536, 100
D_CP, D_FIN, HID = 2, 3, 12
NCORES = 8
BC = B // NCORES              # 8192 per core
G = 7                         # trajectories packed per matmul (block-diag)
C = 3                         # independent column chains
COLS = 391                    # batch columns per chain
BP = G * C * COLS             # 8211 padded batch per core
NH = HID * G                  # 84: h rows (rhs) / pre rows (psum)
NFIN = D_FIN * G              # 21 fin rows
NCONST = 1 + D_CP * G         # 15: ones + cp0 rows
KTOT = NH + NFIN + NCONST     # 120
MOUT = D_CP * G               # 14 out rows
MPAD = 96                     # out region starts at psum partition 96
MTOT = MPAD + MOUT            # 110
NFG = (T + 3) // 4 + 1        # 26 fin groups (steps 4g..4g+3, zero padded)

F32 = mybir.dt.float32

_MM_CHOICES = {"bf16": mybir.dt.bfloat16, "f32r": mybir.dt.float32r, "f32": F32}
MM_DTYPE = _MM_CHOICES[os.environ.get("DTB_MM", "bf16")]
MM_NP = mybir.dt.np(MM_DTYPE)

LAST_RESULTS = None  # test.py introspects profiling info from here


def build_program(t_steps=T, g=G, c=C, cols=COLS, mm_dtype=None):
    if mm_dtype is None:
        mm_dtype = MM_DTYPE
    XDT = mm_dtype
    nh, nfin, nconst = HID * g, D_FIN * g, 1 + D_CP * g
    ktot = nh + nfin + nconst
    mout = D_CP * g
    mpad, mtot = MPAD, MPAD + D_CP * g
    nfg = (t_steps + 3) // 4 + 1
    nog = t_steps // 4
    nc = bacc.Bacc(target_bir_lowering=False)

    fin = nc.dram_tensor("fin", [c, nfin, nfg, 4 * cols], XDT, kind="ExternalInput")
    # xinit0 = full rhs block 0 (h_init + fin g0s0 + ones + cp0); xinitR =
    # rows nh: for blocks 1-7.
    xinit0 = nc.dram_tensor("xinit0", [c, ktot, cols], XDT, kind="ExternalInput")
    xinitR = nc.dram_tensor(
        "xinitR", [c, nfin + nconst, 7 * cols], XDT, kind="ExternalInput"
    )
    w = nc.dram_tensor("w", [ktot, mtot], XDT, kind="ExternalInput")
    out = nc.dram_tensor("out", [nog, c, mout, 4 * cols], XDT, kind="ExternalOutput")

    tanh = mybir.ActivationFunctionType.Tanh

    with tile.TileContext(nc) as tc, ExitStack() as ctx:
        const = ctx.enter_context(tc.tile_pool(name="const", bufs=1))
        xpool = ctx.enter_context(tc.tile_pool(name="xpool", bufs=1))
        opool = ctx.enter_context(tc.tile_pool(name="opool", bufs=3))
        psum = ctx.enter_context(tc.tile_pool(name="psum", bufs=2, space="PSUM"))

        # Critical path to the first matmul: ws first on sync (the earliest-
        # starting queue) + per-chain full block 0, one DMA per queue
        # (h_init rows make the steady-state weights exact at t=0, so there
        # is no w0 variant).
        ws = const.tile([ktot, mtot], XDT, name="ws")
        qs = [nc.gpsimd, nc.scalar, nc.sync]
        nc.sync.dma_start(out=ws, in_=w[:, :])
        xtiles = []
        for ch in range(c):
            xt = xpool.tile([ktot, 8 * cols], XDT, tag=f"x{ch}", name=f"x_{ch}")
            qs[ch].dma_start(out=xt[:, 0:cols], in_=xinit0[ch][:, :])
            xtiles.append(xt)
        # xinitR split by first-use step: block 1 (t=1), blocks 2-3, blocks
        # 4-7 — each queue issues them in deadline order.
        for lo, hi in ((0, 1), (1, 3), (3, 7)):
            for ch in range(c):
                qs[ch].dma_start(
                    out=xtiles[ch][nh:, (lo + 1) * cols : (hi + 1) * cols],
                    in_=xinitR[ch][:, lo * cols : hi * cols],
                )

        # fin prefetch on sync (ch 0/1) + gpsimd (ch 2); out stores on
        # gpsimd (ch 0/1) + sync (ch 2) so neither queue owns both flows.
        # The final-group halves alternate between scalar (whose DMA ring is
        # nearly empty, so the end-of-program drain clears it quickly — the
        # ACT engine is done by then) and sync, so their issue costs overlap.
        finq = [nc.sync, nc.sync, nc.gpsimd]
        outq = [nc.gpsimd, nc.gpsimd, nc.sync]
        lastq = {(0, 1): nc.scalar, (0, 3): nc.sync, (1, 1): nc.sync,
                 (1, 3): nc.scalar, (2, 1): nc.scalar, (2, 3): nc.sync}

        ostages = [None] * c
        for t in range(t_steps + 1):
            for ch in range(c):
                xt = xtiles[ch]
                blk = t % 8
                p1 = psum.tile([mtot, cols], F32, tag=f"p{ch}", name=f"p_{ch}_{t}")
                nc.tensor.matmul(
                    p1, ws,
                    xt[:, blk * cols : (blk + 1) * cols], start=True, stop=True,
                )

                if t < t_steps:
                    # h_t goes straight into the next step's rhs block.
                    # (Splitting this tanh in halves was tried to deepen the
                    # PSUM-access pipelining and measured 213us vs 164us:
                    # the ~100ns back-to-back ACT overlap is a hard cap, so
                    # per-instruction overhead dominates any split.)
                    nb = (t + 1) % 8
                    nc.scalar.activation(
                        xt[0:nh, nb * cols : (nb + 1) * cols], p1[0:nh, :], tanh
                    )
                if t > 0:
                    ob = (t - 1) % 4
                    if ob == 0:
                        ostages[ch] = opool.tile(
                            [mout, 4 * cols], XDT, tag=f"o{ch}", name=f"o_{ch}_{t}"
                        )
                    ost = ostages[ch]
                    if t == t_steps and ch < 2:
                        # Final round: tanh is done, so the idle ACT engine
                        # absorbs two of the three trailing out copies while
                        # the DVE takes the third.
                        nc.scalar.activation(
                            ost[:, ob * cols : (ob + 1) * cols],
                            p1[mpad:mtot, :],
                            mybir.ActivationFunctionType.Copy,
                        )
                    else:
                        nc.vector.tensor_copy(
                            out=ost[:, ob * cols : (ob + 1) * cols],
                            in_=p1[mpad:mtot, :],
                        )
                    final = t > t_steps - 4
                    if final and ob == 1:
                        # Drain the last group in halves so the tail DMA is
                        # small and mostly overlapped with the closing steps.
                        lastq[ch, 1].dma_start(
                            out=out[(t - 2) // 4, ch][:, : 2 * cols],
                            in_=ost[:, : 2 * cols],
                        )
                    elif ob == 3:
                        if final:
                            lastq[ch, 3].dma_start(
                                out=out[(t - 4) // 4, ch][:, 2 * cols :],
                                in_=ost[:, 2 * cols :],
                            )
                        else:
                            outq[ch].dma_start(out=out[(t - 4) // 4, ch], in_=ost)
                # Prefetch fin group g = steps 4g..4g+3, ~5 steps ahead.
                # Groups past the last real step are never prefetched: the
                # t=t_steps matmul only reuses stale fin rows whose products
                # land in discarded pre rows.
                if t % 4 == 3:
                    gg = (t + 5) // 4
                    if 4 * gg < t_steps:
                        b0 = (4 * gg) % 8
                        finq[ch].dma_start(
                            out=xt[nh : nh + nfin, b0 * cols : (b0 + 4) * cols],
                            in_=fin[ch, :, gg, :],
                        )
    if os.environ.get("DTB_SKIPLDW"):
        _skip_repeat_ldweights(nc)
    nc.compile()
    return nc


def _skip_repeat_ldweights(nc):
    """Delete LDWEIGHTS whose weights AP matches the previous one in the PE
    stream (the array still holds those weights).  Waits are consolidated
    onto matmuls/ldweights first so only wait-free repeats are dropped."""
    nc.move_matmul_waits_to_ldweights()
    nc.generate_event_semaphores()
    removed = 0
    for blk in nc.main_func.blocks:
        insts = blk.instructions
        prev_key = None
        idx = 0
        while idx < len(insts):
            inst = insts[idx]
            if isinstance(inst, mybir.InstLdweights):
                key = str(inst.ins[0])
                si = inst.sync_info
                nowait = si is None or (
                    len(si.on_wait) == 0 and len(si.on_update) == 0
                )
                if key == prev_key and nowait:
                    del insts[idx]
                    removed += 1
                    continue
                prev_key = key
            idx += 1
    print(f"skip_ldweights: removed {removed} weight loads")


def build_packed_weights(W_rnn, W_out, b_rnn, b_out, g=G):
    W_rnn = np.asarray(W_rnn, np.float32)
    W_out = np.asarray(W_out, np.float32)
    b_rnn = np.asarray(b_rnn, np.float32)
    b_out = np.asarray(b_out, np.float32)
    W1p, W1f = W_rnn[:D_CP], W_rnn[D_CP:]
    Wo1, Wo2 = W_out[:D_CP], W_out[D_CP:]
    nh, nfin = HID * g, D_FIN * g
    ktot = nh + nfin + 1 + D_CP * g
    mpad, mtot = MPAD, MPAD + D_CP * g
    ones_row = nh + nfin
    cp0_base = ones_row + 1

    w = np.zeros((ktot, mtot), np.float32)
    w0 = np.zeros((ktot, mtot), np.float32)
    E = Wo1 @ W1p                      # (2, 12) cp0 contribution to next pre
    r = b_rnn + b_out @ W1p            # (12,) ones-row weight (steady state)
    Wh = Wo2 @ W1p                     # (12, 12) h contribution to next pre
    for i in range(g):
        hsl = slice(HID * i, HID * (i + 1))
        osl = slice(mpad + D_CP * i, mpad + D_CP * (i + 1))
        w[hsl, hsl] = Wh
        w[hsl, osl] = Wo2
        w0[hsl, osl] = Wo2             # unused at t=0 (h rows are zero) but harmless
        fsl = slice(nh + D_FIN * i, nh + D_FIN * (i + 1))
        w[fsl, hsl] = W1f
        w0[fsl, hsl] = W1f
        w[ones_row, hsl] = r
        w0[ones_row, hsl] = b_rnn
        # cvec = cp0 @ Wo1 + b_out folded into the out columns so psum[96:110]
        # holds the final out and the DVE pass is a pure copy+cast.
        w[ones_row, osl] = b_out
        w0[ones_row, osl] = b_out
        csl = slice(cp0_base + D_CP * i, cp0_base + D_CP * (i + 1))
        w[csl, hsl] = E
        w0[csl, hsl] = W1p
        w[csl, osl] = Wo1
        w0[csl, osl] = Wo1
    return w, w0


def stage_inputs(cp0, fin, h_init, g=G, c=C, cols=COLS, t_steps=T):
    """Batch-major -> feature-major device layouts (b = ch*(g*cols)+gi*cols+j)."""
    bp = g * c * cols
    bc = cp0.shape[0]
    nfg = (t_steps + 3) // 4 + 1
    F = np.zeros((bp, 4 * nfg, D_FIN), np.float32)
    F[:bc, :t_steps] = fin
    cp0_p = np.zeros((bp, D_CP), np.float32)
    cp0_p[:bc] = cp0
    hi_p = np.zeros((bp, HID), np.float32)
    hi_p[:bc] = h_init
    # fin group g covers steps 4g..4g+3, contiguous per partition row.
    fin_d = np.ascontiguousarray(
        F.reshape(c, g, cols, nfg, 4, D_FIN).transpose(0, 1, 5, 3, 4, 2)
    ).reshape(c, D_FIN * g, nfg, 4 * cols)
    nh, nfin, nconst = HID * g, D_FIN * g, 1 + D_CP * g
    cp0_rows = cp0_p.reshape(c, g, cols, D_CP).transpose(0, 1, 3, 2).reshape(
        c, D_CP * g, cols
    )
    h_rows = hi_p.reshape(c, g, cols, HID).transpose(0, 1, 3, 2).reshape(
        c, nh, cols
    )
    x0 = np.ones((c, nh + nfin + nconst, cols), np.float32)
    x0[:, :nh] = h_rows
    x0[:, nh : nh + nfin] = fin_d[:, :, 0, 0:cols]
    x0[:, nh + nfin + 1 :] = cp0_rows
    xr = np.ones((c, nfin + nconst, 7 * cols), np.float32)
    xr[:, :nfin] = fin_d[:, :, 0:2, :].reshape(c, nfin, 8 * cols)[:, :, cols:]
    xr[:, nfin + 1 :] = np.tile(cp0_rows, (1, 1, 7))
    return fin_d, x0, xr


def unstage_output(out_d, bc, g=G, c=C, cols=COLS, t_steps=T):
    """out_d [T/4, c, 14, 4*cols] (step-within-group along the row) -> batch."""
    bp = g * c * cols
    nog = t_steps // 4
    o = out_d.reshape(nog, c, g, D_CP, 4, cols).transpose(1, 2, 5, 0, 4, 3)
    return np.ascontiguousarray(o).reshape(bp, t_steps, D_CP)[:bc]


# ---------------------------------------------------------------------------
# v2: C=4 chains, rotating ACT pairs (2-bank PSUM APs), single quad-bank DVE
# copy per step, one 8-bank PSUM mega tile, no w0 (h rows of block 0 start at
# h_init = (cp0 - cvec) @ pinv(Wo2) so the steady-state weights are exact at
# t=0 too).
#
# Bank map per round t: even t -> banks 0-3 hold chains (0,1,2,3); odd t ->
# banks 4-7 hold chains (0,2,1,3).  ACT pair X always reads banks
# (base,base+1), Y (base+2,base+3); each next-round ACT pair waits on two
# matmuls fed by two DIFFERENT prior ACTs, so the serial cycle is
# ACT+1*MM+2*sem instead of ACT+2*MM+2*sem.
# rhs block layout: even blocks [ch0|ch2|ch1|ch3], odd blocks [ch0|ch1|ch2|ch3]
# (block parity == step parity), which makes every ACT output a contiguous
# half-block.
# ---------------------------------------------------------------------------

C2 = 4
COLS2 = 293                   # 7*4*293 = 8204 >= 8192
W2 = C2 * COLS2               # 1172 block width
BP2 = G * C2 * COLS2
PERM_EVEN = [0, 2, 1, 3]      # chain occupying each slot, even blocks
PERM_ODD = [0, 1, 2, 3]


def build_program2(t_steps=T, mm_dtype=None):
    if mm_dtype is None:
        mm_dtype = MM_DTYPE
    XDT = mm_dtype
    g, cols = G, COLS2
    nh, nfin, nconst = HID * g, D_FIN * g, 1 + D_CP * g
    ktot = nh + nfin + nconst
    mout = D_CP * g
    mpad, mtot = MPAD, MPAD + mout
    nfg = (t_steps + 3) // 4 + 1
    nog = t_steps // 4
    nc = bacc.Bacc(target_bir_lowering=False)

    fin = nc.dram_tensor("fin", [nfin, nfg, 4 * W2], XDT, kind="ExternalInput")
    xinit0 = nc.dram_tensor("xinit0", [ktot, W2], XDT, kind="ExternalInput")
    xinitR = nc.dram_tensor(
        "xinitR", [nfin + nconst, 7 * W2], XDT, kind="ExternalInput"
    )
    w = nc.dram_tensor("w", [ktot, mtot], XDT, kind="ExternalInput")
    out = nc.dram_tensor("out", [nog, mout, 4 * W2], XDT, kind="ExternalOutput")

    tanh = mybir.ActivationFunctionType.Tanh

    with tile.TileContext(nc) as tc, ExitStack() as ctx:
        const = ctx.enter_context(tc.tile_pool(name="const", bufs=1))
        xpool = ctx.enter_context(tc.tile_pool(name="xpool", bufs=1))
        opool = ctx.enter_context(tc.tile_pool(name="opool", bufs=3))
        psum = ctx.enter_context(tc.tile_pool(name="psum", bufs=1, space="PSUM"))

        ws = const.tile([ktot, mtot], XDT, name="ws")
        xt = xpool.tile([ktot, 8 * W2], XDT, name="xt")
        # Separate 4-bank tiles for even/odd rounds keep the dependency
        # tracker's ranges from uniting across parities.
        pmA = psum.tile([mtot, 2048], F32, name="pmA")
        pmB = psum.tile([mtot, 2048], F32, name="pmB")

        # Setup: 4 DMAs. Critical: block 0 (sync) + weights (gpsimd);
        # blocks 1-3 next (needed at t=1), 4-7 last (scalar starts late
        # behind its ACT_TABLE_LOAD anyway).
        nc.sync.dma_start(out=xt[:, 0:W2], in_=xinit0[:, :])
        nc.gpsimd.dma_start(out=ws, in_=w[:, :])
        nc.gpsimd.dma_start(
            out=xt[nh:, W2 : 4 * W2], in_=xinitR[:, 0 : 3 * W2]
        )
        nc.scalar.dma_start(out=xt[nh:, 4 * W2 :], in_=xinitR[:, 3 * W2 :])

        pm3A = pmA[:, :].rearrange("p (b q) -> p b q", q=512)
        pm3B = pmB[:, :].rearrange("p (b q) -> p b q", q=512)

        ostage = None
        for t in range(t_steps + 1):
            even = t % 2 == 0
            pm = pmA if even else pmB
            pm3 = pm3A if even else pm3B
            bank_ch = PERM_ODD if even else PERM_EVEN   # bank slot -> chain
            pblk = PERM_EVEN if even else PERM_ODD      # rhs block slot -> chain
            rs = [pblk.index(bank_ch[s]) for s in range(4)]
            blk = t % 8
            nb = (t + 1) % 8

            def mm(s):
                nc.tensor.matmul(
                    pm[:, 512 * s : 512 * s + cols],
                    ws,
                    xt[:, blk * W2 + rs[s] * cols : blk * W2 + (rs[s] + 1) * cols],
                    start=True, stop=True,
                )

            def act(half):
                nc.scalar.activation(
                    xt[0:nh, nb * W2 + half * 2 * cols : nb * W2 + (half + 1) * 2 * cols]
                    .rearrange("p (b q) -> p b q", q=cols),
                    pm3[0:nh, 2 * half : 2 * half + 2, 0:cols],
                    tanh,
                )

            mm(0)
            mm(1)
            if t < t_steps:
                act(0)
            mm(2)
            mm(3)
            if t < t_steps:
                act(1)
            if t > 0:
                ob = (t - 1) % 4
                if ob == 0:
                    ostage = opool.tile([mout, 4 * W2], XDT, tag="o", name=f"o_{t}")
                nc.vector.tensor_copy(
                    out=ostage[:, ob * W2 : (ob + 1) * W2]
                    .rearrange("p (b q) -> p b q", q=cols),
                    in_=pm3[mpad:mtot, 0:4, 0:cols],
                )
                final = t > t_steps - 4
                if final and ob == 1:
                    nc.scalar.dma_start(
                        out=out[(t - 2) // 4][:, : 2 * W2],
                        in_=ostage[:, : 2 * W2],
                    )
                elif ob == 3:
                    if final:
                        nc.scalar.dma_start(
                            out=out[(t - 4) // 4][:, 2 * W2 :],
                            in_=ostage[:, 2 * W2 :],
                        )
                    else:
                        nc.gpsimd.dma_start(out=out[(t - 4) // 4], in_=ostage)
            if t % 4 == 3:
                gg = (t + 5) // 4
                if 4 * gg < t_steps:
                    b0 = (4 * gg) % 8
                    nc.sync.dma_start(
                        out=xt[nh : nh + nfin, b0 * W2 : (b0 + 4) * W2],
                        in_=fin[:, gg, :],
                    )
    if os.environ.get("DTB_SKIPLDW"):
        _skip_repeat_ldweights(nc)
    nc.compile()
    return nc


def stage_inputs2(cp0, fin, h_init, t_steps=T):
    """Per-core staging for v2 (chain-rotated block layouts)."""
    g, c, cols = G, C2, COLS2
    bp, bc = BP2, cp0.shape[0]
    nfg = (t_steps + 3) // 4 + 1
    F = np.zeros((bp, 4 * nfg, D_FIN), np.float32)
    F[:bc, :t_steps] = fin
    cp0_p = np.zeros((bp, D_CP), np.float32)
    cp0_p[:bc] = cp0
    hi_p = np.zeros((bp, HID), np.float32)
    hi_p[:bc] = h_init

    # feature-major per chain: [c, rows, ..., cols]
    ffeat = F.reshape(c, g, cols, nfg, 4, D_FIN).transpose(0, 1, 5, 3, 4, 2)
    ffeat = np.ascontiguousarray(ffeat).reshape(c, D_FIN * g, nfg, 4, cols)
    cfeat = cp0_p.reshape(c, g, cols, D_CP).transpose(0, 1, 3, 2).reshape(
        c, D_CP * g, cols
    )
    hfeat = hi_p.reshape(c, g, cols, HID).transpose(0, 1, 3, 2).reshape(
        c, HID * g, cols
    )

    nfin = D_FIN * g
    fin_d = np.zeros((nfin, nfg, 4, c * cols), np.float32)
    for s in range(4):
        perm = PERM_EVEN if s % 2 == 0 else PERM_ODD
        for slot in range(c):
            fin_d[:, :, s, slot * cols : (slot + 1) * cols] = ffeat[
                perm[slot], :, :, s, :
            ]
    cp_rows = np.zeros((2, D_CP * g, c * cols), np.float32)  # [parity]
    for par in range(2):
        perm = PERM_EVEN if par == 0 else PERM_ODD
        for slot in range(c):
            cp_rows[par, :, slot * cols : (slot + 1) * cols] = cfeat[perm[slot]]

    ktot = HID * g + nfin + 1 + D_CP * g
    x0 = np.ones((ktot, c * cols), np.float32)
    for slot in range(c):
        x0[: HID * g, slot * cols : (slot + 1) * cols] = hfeat[PERM_EVEN[slot]]
    x0[HID * g : HID * g + nfin] = fin_d[:, 0, 0, :]
    x0[HID * g + nfin + 1 :] = cp_rows[0]

    xr = np.ones((nfin + 1 + D_CP * g, 7, c * cols), np.float32)
    for b in range(1, 8):
        xr[:nfin, b - 1] = fin_d[:, b // 4, b % 4, :]
        xr[nfin + 1 :, b - 1] = cp_rows[b % 2]
    xr = xr.reshape(-1, 7 * c * cols)
    fin_d = fin_d.reshape(nfin, nfg, 4 * c * cols)
    return fin_d, x0, xr


def unstage_output2(out_d, bc, t_steps=T):
    """out_d [nog, 14, 4, c*cols] -> [bc, T, 2].  Sub-block ob holds step
    tau=4q+ob computed at round t=tau+1; chain order is identity for odd tau,
    [0,2,1,3] for even tau."""
    g, c, cols = G, C2, COLS2
    nog = t_steps // 4
    o = out_d.reshape(nog, D_CP * g, 4, c, cols)
    res = np.empty((c, g, cols, nog, 4, D_CP), np.float32)
    for ob in range(4):
        perm = PERM_EVEN if ob % 2 == 0 else PERM_ODD
        for slot in range(c):
            # o[q, 2*gi+d, ob, slot, j] -> chain perm[slot]
            res[perm[slot], :, :, :, ob, :] = o[:, :, ob, slot, :].reshape(
                nog, g, D_CP, cols
            ).transpose(1, 3, 0, 2)
    return res.reshape(BP2, t_steps, D_CP)[:bc]


def kernel(control_point_input, finger_input, W_rnn, U_rnn, b_rnn, W_out, b_out):
    global LAST_RESULTS
    cp = np.asarray(control_point_input, np.float32)
    fin = np.asarray(finger_input, np.float32)
    W_rnn = np.asarray(W_rnn, np.float32)
    b_rnn = np.asarray(b_rnn, np.float32)
    W_out = np.asarray(W_out, np.float32)
    b_out = np.asarray(b_out, np.float32)

    cp0 = cp[:, 0, :]
    w, _ = build_packed_weights(W_rnn, W_out, b_rnn, b_out)
    w = w.astype(MM_NP)
    # h_init makes the steady-state weights exact at t=0 (no w0 pass needed):
    # h_init @ Wo2 = cp0 - cvec reproduces out_{-1} = cp0 through the packed
    # recurrence.
    Wo2 = W_out[D_CP:]
    cvec = cp0 @ W_out[:D_CP] + b_out
    h_init = (cp0 - cvec) @ np.linalg.pinv(Wo2)

    trace = bool(os.environ.get("DTB_TRACE"))
    if os.environ.get("DTB_V2"):
        nc = build_program2()
        in_maps = []
        for m in range(NCORES):
            sl = slice(m * BC, (m + 1) * BC)
            fin_d, x0_d, xr_d = stage_inputs2(cp0[sl], fin[sl], h_init[sl])
            in_maps.append(
                {"fin": fin_d.astype(MM_NP, copy=False),
                 "xinit0": x0_d.astype(MM_NP, copy=False),
                 "xinitR": xr_d.astype(MM_NP, copy=False), "w": w}
            )
        res = run_bass_kernel_spmd(
            nc, in_maps, core_ids=list(range(NCORES)), trace=trace
        )
        LAST_RESULTS = res
        outs = [
            unstage_output2(np.asarray(res.results[m]["out"], np.float32), BC)
            for m in range(NCORES)
        ]
        return np.concatenate(outs, axis=0)

    nc = build_program()
    in_maps = []
    for m in range(NCORES):
        sl = slice(m * BC, (m + 1) * BC)
        fin_d, x0_d, xr_d = stage_inputs(cp0[sl], fin[sl], h_init[sl])
        in_maps.append(
            {"fin": fin_d.astype(MM_NP, copy=False),
             "xinit0": x0_d.astype(MM_NP, copy=False),
             "xinitR": xr_d.astype(MM_NP, copy=False), "w": w}
        )
    res = run_bass_kernel_spmd(
        nc, in_maps, core_ids=list(range(NCORES)), trace=trace
    )
    LAST_RESULTS = res
    outs = [
        unstage_output(np.asarray(res.results[m]["out"], np.float32), BC)
        for m in range(NCORES)
    ]
    return np.concatenate(outs, axis=0)

